# revision 1
# baseline (speedup 1.0000x reference)
"""Sparse cosine-similarity attention kernel for Trainium2 (8 NeuronCores).

Problem: query [16,16,1,128], key [16,16,4096,128], mask [16,4096] int32
  scores[b,h,l] = <q,k_l> / max(|q||k_l|, 1e-8);  masked softmax over l.
Output: p_attn [16,16,4096] float32;  p_attn[b,h,l] = 0 where mask[b,l] = 0.

Sharding: batch dim split across 8 cores (2 batches/core, 32 (b,h) rows).

Sparsity: masked keys (~50%) never influence the output, so the host
compacts, per batch b, the kept key indices L_b = {l : mask[b,l]=1} (padded
to NK with index 0; padding killed later by padmask) and the kernel gathers
ONLY those K rows via SWDGE dma_gather — roughly halving both HBM traffic
and compute versus the dense kernel.

Per-core dataflow (compact position i = c*128 + p, chunk c, partition p):
  - dma_gather per (b,h): K[b,h,L_b,:] -> slab [128(p), NK/128(c), 128(d)]
  - PE transposes (fp32r) -> PSUM KT blocks [128(d), 512(i)]
  - DVE copy-drain  -> KT  (fp32r), ACT square-drain -> K2T (fp32r)
  - dots:  masked-Q accumulate matmuls -> psum rows 0-31  of fused bank
  - norms: masked-1s accumulate matmuls -> psum rows 32-63 (tile_position)
    Both accumulate over all 32 (b,h) as slabs stream through.
  - tail per block: rk = exp(-0.5*ln(qn2*kn2)); e = exp(dots*rk)*padmask
  - softmax normalize in compact space, then gpsimd.ap_gather scatters the
    compact probabilities back to dense l-order (host-built inverse indices;
    trash slot NK = 0.0 supplies the masked zeros), 512-col chunk stores.

softmax max-subtraction is dropped: scores are cosine similarities in [-1,1].
"""

import sys

if "/opt/trn_rl_repo" not in sys.path:
    sys.path.insert(0, "/opt/trn_rl_repo")

import numpy as np

import concourse.bacc as bacc
import concourse.tile as tile
from concourse import mybir
from concourse.bass_utils import run_bass_kernel_spmd
from concourse.masks import make_identity

F32 = mybir.dt.float32
F32R = mybir.dt.float32r
I16 = mybir.dt.int16
AF = mybir.ActivationFunctionType
AX = mybir.AxisListType

B, H, L, D = 16, 16, 4096, 128
NCORES = 8
BLOC = B // NCORES  # batches per core
NBH = BLOC * H  # 32 (b,h) rows per core
LB = 512  # max block size (psum bank = 512 fp32 per partition)

_ONE_SET = "natural_log_exp_and_others"  # contains Copy/Identity/Square/Ln/Exp


class _Bacc(bacc.Bacc):
    """Bacc that pins all activations to a single ACT table set, avoiding
    ~2.7us table reloads when Square and Ln/Exp interleave."""

    PIN_TABLES = True

    def insert_act_table_loads(self):
        super().insert_act_table_loads()
        if not self.PIN_TABLES:
            return
        from concourse.hw_specs import get_activation_tables

        names = list(get_activation_tables(self.m.arch).keys())
        target = names.index(_ONE_SET)
        first = True
        for fn in self.m.functions:
            for blk in fn.blocks:
                keep = []
                changed = False
                for inst in blk.instructions:
                    if type(inst).__name__ == "InstLoadActFuncSet":
                        if first:
                            inst.act_func_set_id = target
                            first = False
                            keep.append(inst)
                        else:
                            changed = True
                        continue
                    keep.append(inst)
                if changed:
                    del blk.instructions[:]
                    for i in keep:
                        blk.instructions.append(i)


def _blocks(nk):
    """Split nk compact columns into PSUM-bank blocks of <= LB columns."""
    out = []
    c = 0
    while c < nk:
        sz = min(LB, nk - c)
        out.append((c, sz))
        c += sz
    return out


def build_module(nk, variant="full", reps=1):
    assert nk % 128 == 0
    nchunk = nk // 128
    blocks = _blocks(nk)
    nblk = len(blocks)
    nc = _Bacc(
        "TRN2",
        target_bir_lowering=False,
        debug=False,
        num_devices=NCORES,
        num_swdge_queues=1,
    )
    q_d = nc.dram_tensor("query", [BLOC, H, 1, D], F32, kind="ExternalInput").ap()
    k_d = nc.dram_tensor("key", [BLOC, L, H, D], F32, kind="ExternalInput").ap()
    idx_d = nc.dram_tensor(
        "kidx", [BLOC, 128, nk // 16], I16, kind="ExternalInput"
    ).ap()
    inv_d = nc.dram_tensor("kinv", [NBH, L // 16], I16, kind="ExternalInput").ap()
    pm_d = nc.dram_tensor("padmask", [NBH, nk], F32, kind="ExternalInput").ap()
    o_d = nc.dram_tensor("out", [BLOC, H, L], F32, kind="ExternalOutput").ap()

    with tile.TileContext(nc) as tc:
        with (
            tc.tile_pool(name="consts", bufs=1) as consts,
            tc.tile_pool(name="persist", bufs=1) as pers,
            tc.tile_pool(name="gpool", bufs=3) as gpool,
            tc.tile_pool(name="ktp", bufs=10) as ktp,
            tc.tile_pool(name="k2tp", bufs=10) as k2tp,
            tc.tile_pool(name="pst", bufs=4, space="PSUM") as pst,
            tc.tile_pool(name="psdp", bufs=2, space="PSUM") as psdp,
            tc.tile_pool(name="psnp", bufs=2, space="PSUM") as psnp,
        ):
            # ---------------- prologue: constants -----------------
            ident = consts.tile([128, 128], F32)
            make_identity(nc, ident)
            identr = consts.tile([128, 128], F32R)
            nc.scalar.copy(identr[:], ident[:])

            qsb = pers.tile([NBH, D], F32, tag="qsb")
            nc.sync.dma_start(qsb[:], q_d.rearrange("b h o d -> (b h) (o d)"))

            # qn2[bh] = |q_bh|^2  (fused square+reduce on DVE)
            junkq = pers.tile([NBH, D], F32, tag="junkq")
            qn2 = pers.tile([NBH, 1], F32, tag="qn2")
            nc.vector.scalar_tensor_tensor(
                out=junkq[:],
                in0=qsb[:],
                scalar=1.0,
                in1=qsb[:],
                op0=mybir.AluOpType.mult,
                op1=mybir.AluOpType.mult,
                accum_out=qn2[:],
            )

            # qt [128(d), 32(bh)]
            qt_ps = psdp.tile([128, NBH], F32, tag="d", name="qt_ps")
            nc.tensor.transpose(qt_ps[:], qsb[:], ident[0:NBH, 0:NBH])
            qt = pers.tile([128, NBH], F32, tag="qt")
            nc.scalar.copy(qt[:], qt_ps[:])

            # masked stationaries (fp32r, all ACT-produced):
            # MQ[:, bh, :] has q_bh in column bh, zeros elsewhere.
            # MONES[:, bh, :] has ones in column bh.
            mq = pers.tile([128, NBH, NBH], F32R, tag="mq")
            nc.scalar.activation(
                mq[:],
                qt[:].unsqueeze(1).broadcast_to([128, NBH, NBH]),
                AF.Copy,
                scale=0.0,
            )
            mones = pers.tile([128, NBH, NBH], F32R, tag="mones")
            nc.scalar.activation(
                mones[:],
                qt[:].unsqueeze(1).broadcast_to([128, NBH, NBH]),
                AF.Copy,
                scale=0.0,
            )
            for bh in range(NBH):
                nc.scalar.copy(mq[:, bh, bh : bh + 1], qt[:, bh : bh + 1])
                nc.scalar.activation(
                    mones[:, bh, bh : bh + 1],
                    qt[:, 0:1],
                    AF.Copy,
                    bias=1.0,
                    scale=0.0,
                )

            # gather indices (int16, wrapped [i%16, i//16], replicated over
            # the 8 groups of 16 partitions), one tile per local batch
            idxs = []
            for b in range(BLOC):
                idx_b = pers.tile([128, nk // 16], I16, tag=f"idx{b}", name=f"idx{b}")
                nc.sync.dma_start(idx_b[:], idx_d[b])
                idxs.append(idx_b)
            inv_sb = pers.tile([NBH, L // 16], I16, tag="inv")
            nc.sync.dma_start(inv_sb[:], inv_d)
            pmask = pers.tile([NBH, nk], F32, tag="pmask")
            nc.scalar.dma_start(pmask[:], pm_d)

            scomp = pers.tile([NBH, nk + 1], F32, tag="scomp")
            nc.vector.memset(scomp[:], 0.0)  # incl. ap_gather trash slot at nk
            kn2d = pers.tile([NBH, nk], F32, tag="kn2d")
            partials = pers.tile([NBH, nblk], F32, tag="partials")
            dense = pers.tile([NBH, L], F32, tag="dense")

            # ---------------- main loop -----------------
            # One gather per (b, block): 8KB descriptors cover all 16 heads
            # of a kept l (key is host-transposed to [B, L, H, D]).  Block-
            # major order keeps exactly one dots/norms PSUM pair alive.
            def one_pass():
                def _mms(bh, kt, k2t, sz):
                    nc.tensor.matmul(
                        pbd[:, 0:sz],
                        mq[:, bh, :],
                        kt[:],
                        start=(bh == 0),
                        stop=(bh == NBH - 1),
                        skip_group_check=True,
                    )
                    nc.tensor.matmul(
                        pbn[:, 0:sz],
                        mones[:, bh, :],
                        k2t[:],
                        start=(bh == 0),
                        stop=(bh == NBH - 1),
                        skip_group_check=True,
                    )

                def dense_chunk(j):
                    # dense l-chunk j reads compact positions < (j+1)*LB only
                    sl = slice(j * LB, (j + 1) * LB)
                    if variant == "noapg":
                        nc.vector.memset(dense[:, sl], 0.0)
                        return
                    nc.gpsimd.ap_gather(
                        dense[:, sl].unsqueeze(2),
                        scomp[:].unsqueeze(2),
                        inv_sb[:, j * (LB // 16) : (j + 1) * (LB // 16)],
                        NBH,
                        nk + 1,
                        1,
                        LB,
                    )

                def epilogue(j):
                    c0, sz = blocks[j]
                    sl = slice(c0, c0 + sz)
                    nc.vector.tensor_copy(scomp[:, sl], pbd[:, 0:sz])
                    nc.scalar.copy(kn2d[:, sl], pbn[:, 0:sz])
                    nc.vector.tensor_scalar_mul(kn2d[:, sl], kn2d[:, sl], qn2[:])
                    nc.scalar.activation(kn2d[:, sl], kn2d[:, sl], AF.Ln)
                    nc.scalar.activation(
                        kn2d[:, sl], kn2d[:, sl], AF.Exp, scale=-0.5
                    )
                    nc.vector.tensor_mul(scomp[:, sl], scomp[:, sl], kn2d[:, sl])
                    nc.scalar.activation(scomp[:, sl], scomp[:, sl], AF.Exp)
                    # e *= padmask, with per-row partial sums (one DVE op)
                    nc.vector.scalar_tensor_tensor(
                        out=scomp[:, sl],
                        in0=scomp[:, sl],
                        scalar=1.0,
                        in1=pmask[:, sl],
                        op0=mybir.AluOpType.mult,
                        op1=mybir.AluOpType.mult,
                        accum_out=partials[:, j : j + 1],
                    )

                for j, (c0, sz) in enumerate(blocks):
                    pbd = psdp.tile([32, LB], F32, tag="d", name="pbd")
                    pbn = psnp.tile([32, LB], F32, tag="n", name="pbn")
                    for b in range(BLOC):
                        slab = gpool.tile(
                            [128, LB // 128, H * D],
                            F32R,
                            tag="slab",
                            name="slab",
                        )
                        nc.gpsimd.dma_gather(
                            slab[:, 0 : sz // 128, :],
                            k_d[b].rearrange("l h d -> l (h d)").bitcast(F32R),
                            idxs[b][:, c0 // 16 : (c0 + sz) // 16],
                            sz,
                            sz,
                            H * D,
                            queue_num=0,
                            single_packet=False,
                        )
                        if variant == "gatheronly":
                            if j == nblk - 1 and b == BLOC - 1:
                                nc.vector.memset(dense[:], 0.0)
                                nc.vector.tensor_add(
                                    dense[:, 0:D],
                                    dense[:, 0:D],
                                    slab[0:NBH, 0, 0:D].bitcast(F32),
                                )
                            continue
                        # software pipeline: head h's accumulate matmuls run
                        # two transpose-sets later (hides kt drain latency)
                        pend = []
                        for h in range(H):
                            bh = b * H + h
                            ng = sz // 128
                            pt = pst.tile([128, sz], F32R, tag="pt", name="pt")
                            for g in range(ng):
                                nc.tensor.matmul(
                                    pt[:, g * 128 : (g + 1) * 128],
                                    slab[:, g, h * D : (h + 1) * D],
                                    identr[:],
                                    is_transpose=True,
                                )
                            kt = ktp.tile([128, sz], F32R, tag="kt", name="kt")
                            nc.vector.tensor_copy(kt[:], pt[:].bitcast(F32))
                            k2t = k2tp.tile(
                                [128, sz], F32R, tag="k2t", name="k2t"
                            )
                            nc.scalar.activation(
                                k2t[:], pt[:].bitcast(F32), AF.Square
                            )
                            pend.append((bh, kt, k2t, sz))
                            if len(pend) > 4 and variant != "nomm":
                                _mms(*pend.pop(0))
                        if variant != "nomm":
                            for p in pend:
                                _mms(*p)
                    if variant not in ("gatheronly",):
                        epilogue(j)
                        if j < min(nblk - 1, L // LB):
                            dense_chunk(j)

                if variant == "gatheronly":
                    nc.sync.dma_start(
                        o_d.rearrange("b h l -> (b h) l"), dense[:]
                    )
                    return

                # ---------------- normalize + dense scatter ----------
                tot = pers.tile([NBH, 1], F32, tag="tot", name="tot")
                nc.vector.reduce_sum(tot[:], partials[:], axis=AX.X)
                srec = pers.tile([NBH, 1], F32, tag="srec", name="srec")
                nc.vector.reciprocal(srec[:], tot[:])

                for j in range(nblk - 1, L // LB):
                    dense_chunk(j)
                for j in range(L // LB):
                    sl = slice(j * LB, (j + 1) * LB)
                    nc.vector.tensor_scalar_mul(dense[:, sl], dense[:, sl], srec[:])
                    nc.sync.dma_start(
                        o_d.rearrange("b h l -> (b h) l")[:, sl], dense[:, sl]
                    )

            if reps == 1:
                one_pass()
            else:
                with tc.For_i(0, reps, 1):
                    one_pass()

    nc.compile()
    return nc


_CACHE = {}


def _get_module(nk, variant="full"):
    key = (nk, variant)
    if key not in _CACHE:
        _CACHE[key] = build_module(nk, variant)
    return _CACHE[key]


def _round_up(x, m):
    return (x + m - 1) // m * m


def _pick_nk(mask):
    counts = (np.asarray(mask) != 0).sum(axis=1)
    return max(_round_up(int(counts.max()), 256), 512)


def _make_in_maps(query, key, mask, nk):
    query = np.asarray(query, np.float32)
    key = np.asarray(key, np.float32)
    mask = np.asarray(mask)
    in_maps = []
    for c in range(NCORES):
        b0 = c * BLOC
        idx = np.zeros((BLOC, 128, nk // 16), np.int16)
        inv = np.zeros((NBH, L // 16), np.int16)
        pm = np.zeros((NBH, nk), np.float32)
        for bl in range(BLOC):
            kept = np.flatnonzero(mask[b0 + bl]).astype(np.int64)
            nb = len(kept)
            assert nb <= nk, f"kept count {nb} exceeds NK {nk}"
            flat = np.zeros(nk, np.int16)  # pad with row 0 (full-valid)
            flat[:nb] = kept.astype(np.int16)
            wrapped = flat.reshape(nk // 16, 16).T  # [16, nk/16]
            idx[bl] = np.tile(wrapped, (8, 1))
            invf = np.full(L, nk, np.int16)  # trash slot
            invf[kept] = np.arange(nb, dtype=np.int16)
            inv[bl * 16 : (bl + 1) * 16] = invf.reshape(L // 16, 16).T
            pm[bl * H : (bl + 1) * H, :nb] = 1.0
        in_maps.append(
            {
                "query": np.ascontiguousarray(query[b0 : b0 + BLOC]),
                "key": np.ascontiguousarray(
                    key[b0 : b0 + BLOC].transpose(0, 2, 1, 3)
                ),
                "kidx": idx,
                "kinv": inv,
                "padmask": pm,
            }
        )
    return in_maps


def _run(query, key, mask, trace=False, nk=None):
    if nk is None:
        nk = _pick_nk(mask)
    nc = _get_module(nk)
    in_maps = _make_in_maps(query, key, mask, nk)
    res = run_bass_kernel_spmd(
        nc, in_maps, core_ids=list(range(NCORES)), trace=trace
    )
    out = np.concatenate([r["out"] for r in res.results], axis=0)
    return out, res


def kernel(query, key, mask):
    out, _ = _run(np.asarray(query), np.asarray(key), np.asarray(mask))
    return out

